# revision 17
# baseline (speedup 1.0000x reference)
"""AdaptiveCenterLoss on 8 TRN2 NeuronCores.

loss = sum((data - cen[labels])**2) / BATCH

Data-parallel over batch: each core handles 8192 rows, gathers its
center rows from a replicated `cen` table via indirect DMA (the
embedding lookup), computes (data-center)^2 with DVE subtract + ACT
square (fused row-sum accumulator), and DMAs per-partition partial
sums out; the host sums partials across partitions/cores (unshard).

Per-core layout (host-prepped):
  data   [128, 64*256]  partition p holds batch rows 64p..64p+63
  labels [128, 64]      labels[p, j] = label of batch row 64p + j
  cen    [100000, 256]  replicated

Tiling: column chunks of K_LIST[t] rows per partition (default 8 tiles
of 8 rows -> 1MB data DMA + 1MB gather per tile), triple-buffered so
the 16 SDMA engines stay saturated; measured at the chip HBM floor
(~2.3 TB/s aggregate for this 128MB working set).
"""

import os

import numpy as np

BATCH = 65536
DIM = 256
NUM_CLASSES = 100000
N_CORES = 8
B_CORE = BATCH // N_CORES  # 8192

P = 128               # SBUF partitions
R = B_CORE // P       # rows per partition (64)

# Small first tiles: each DMA ring is FIFO, so a small tile 0 completes
# early, starting the compute pipeline (and compute-paced DMA issue) sooner.
_klist_env = os.environ.get("ACL_KLIST", "2,6,8,8,8,8,8,8,8")
K_LIST = [int(x) for x in _klist_env.split(",")]
assert sum(K_LIST) == R, K_LIST
NT = len(K_LIST)
BUFS = int(os.environ.get("ACL_BUFS", "3"))

IMPL = os.environ.get("ACL_IMPL", "tile")

_cached = {}


def _build_graph_raw():
    """Raw bacc pipeline: manual semaphores, no Tile prologue/epilogue."""
    from concourse import bass, bacc, mybir

    assert len(set(K_LIST)) == 1, "raw impl assumes uniform tiling"
    k = K_LIST[0]
    B = min(BUFS, NT)

    nc = bacc.Bacc(
        "TRN2",
        target_bir_lowering=False,
        debug=False,
        num_devices=N_CORES,
    )
    f32 = mybir.dt.float32
    i32 = mybir.dt.int32

    data_t = nc.dram_tensor("data", [P, R * DIM], f32, kind="ExternalInput")
    lab_t = nc.dram_tensor("labels", [P, R], i32, kind="ExternalInput")
    cen_t = nc.dram_tensor("cen", [NUM_CLASSES, DIM], f32, kind="ExternalInput")
    out_t = nc.dram_tensor("out", [P, NT], f32, kind="ExternalOutput")

    labs = nc.alloc_sbuf_tensor("labs", [P, R], i32)
    parts = nc.alloc_sbuf_tensor("parts", [P, NT], f32)
    bias = nc.alloc_sbuf_tensor("bias", [P, 1], f32)
    ctrs = [nc.alloc_sbuf_tensor(f"ctr{b}", [P, k * DIM], f32) for b in range(B)]
    dats = [nc.alloc_sbuf_tensor(f"dat{b}", [P, k * DIM], f32) for b in range(B)]

    with (
        nc.Block(no_gpsimd_drain=True) as block,
        nc.semaphore("lab_sem") as lab_sem,
        nc.semaphore("dat_sem") as dat_sem,
        nc.semaphore("ctr_sem") as ctr_sem,
        nc.semaphore("sub_sem") as sub_sem,
        nc.semaphore("sq_sem") as sq_sem,
        nc.semaphore("out_sem") as out_sem,
    ):

        @block.sync
        def _(sync: bass.BassEngine):
            sync.dma_start(out=labs.ap()[:], in_=lab_t.ap()[:]).then_inc(lab_sem, 16)
            for t in range(NT):
                if t >= B:
                    sync.wait_ge(sq_sem, t - B + 1)
                sync.dma_start(
                    out=dats[t % B].ap()[:],
                    in_=data_t.ap()[:, t * k * DIM:(t + 1) * k * DIM],
                ).then_inc(dat_sem, 16)
            sync.wait_ge(sq_sem, NT)
            sync.dma_start(out=out_t.ap()[:], in_=parts.ap()[:]).then_inc(out_sem, 16)
            sync.wait_ge(out_sem, 16)

        @block.gpsimd
        def _(gpsimd: bass.BassEngine):
            gpsimd.wait_ge(lab_sem, 16)
            for t in range(NT):
                if t >= B:
                    gpsimd.wait_ge(sq_sem, t - B + 1)
                gpsimd.indirect_dma_start(
                    out=ctrs[t % B].ap()[:],
                    out_offset=None,
                    in_=cen_t.ap()[:],
                    in_offset=bass.IndirectOffsetOnAxis(
                        ap=labs.ap()[:, t * k:(t + 1) * k], axis=0
                    ),
                ).then_inc(ctr_sem, 16)

        @block.vector
        def _(vector: bass.BassEngine):
            vector.memset(bias.ap()[:], 0.0)
            for t in range(NT):
                vector.wait_ge(dat_sem, 16 * (t + 1))
                vector.wait_ge(ctr_sem, 16 * (t + 1))
                vector.tensor_tensor(
                    out=dats[t % B].ap()[:], in0=dats[t % B].ap()[:],
                    in1=ctrs[t % B].ap()[:],
                    op=mybir.AluOpType.subtract,
                ).then_inc(sub_sem, 1)

        @block.scalar
        def _(scalar: bass.BassEngine):
            for t in range(NT):
                scalar.wait_ge(sub_sem, t + 1)
                scalar.activation(
                    ctrs[t % B].ap()[:], dats[t % B].ap()[:],
                    mybir.ActivationFunctionType.Square,
                    bias=bias.ap()[:, :1],
                    accum_out=parts.ap()[:, t:t + 1],
                ).then_inc(sq_sem, 1)

    nc.compile()
    return nc


def _build_graph():
    if IMPL == "raw":
        return _build_graph_raw()
    from concourse import bass, bacc, mybir, tile

    nc = bacc.Bacc(
        "TRN2",
        target_bir_lowering=False,
        debug=False,
        num_devices=N_CORES,
    )
    f32 = mybir.dt.float32
    i32 = mybir.dt.int32

    # Last tile's compute is chunked so DVE subtract / ACT square pipeline
    # within it, shortening the serial tail after the final input DMA.
    TAIL_CHUNKS = int(os.environ.get("ACL_TAILCHUNKS", "4"))
    n_cols = NT - 1 + TAIL_CHUNKS  # one partial column per compute chunk

    data_t = nc.dram_tensor("data", [P, R * DIM], f32, kind="ExternalInput")
    lab_t = nc.dram_tensor("labels", [P, R], i32, kind="ExternalInput")
    cen_t = nc.dram_tensor("cen", [NUM_CLASSES, DIM], f32, kind="ExternalInput")
    out_t = nc.dram_tensor("out", [P, n_cols], f32, kind="ExternalOutput")

    with tile.TileContext(nc) as tc:
        with (
            tc.tile_pool(name="sbuf", bufs=BUFS) as pool,
            tc.tile_pool(name="persist", bufs=1) as persist,
        ):
            # All gather indices in one small DMA up front.
            labs = persist.tile([P, R], i32)
            nc.sync.dma_start(out=labs[:], in_=lab_t.ap()[:])

            # Per-chunk partial sums in independent columns.
            parts = persist.tile([P, n_cols], f32)

            off = 0
            col = 0
            for t, k in enumerate(K_LIST):
                ctr = pool.tile([P, k * DIM], f32, tag=f"ctr{k}")
                nc.gpsimd.indirect_dma_start(
                    out=ctr[:],
                    out_offset=None,
                    in_=cen_t.ap()[:],
                    in_offset=bass.IndirectOffsetOnAxis(
                        ap=labs[:, off:off + k], axis=0
                    ),
                )

                dat = pool.tile([P, max(K_LIST) * DIM], f32, tag="dat")
                nc.sync.dma_start(
                    out=dat[:, : k * DIM],
                    in_=data_t.ap()[:, off * DIM:(off + k) * DIM],
                )

                # In-place: diff overwrites dat; square's (dead) output
                # overwrites ctr. Keeps live tiles per slot to 2, allowing
                # deeper DMA pipelining via more bufs.
                last = t == len(K_LIST) - 1
                chunks = TAIL_CHUNKS if last and k % TAIL_CHUNKS == 0 else 1
                cw = k * DIM // chunks
                for c in range(chunks):
                    sl = slice(c * cw, (c + 1) * cw)
                    nc.vector.tensor_tensor(
                        out=dat[:, sl], in0=dat[:, sl], in1=ctr[:, sl],
                        op=mybir.AluOpType.subtract,
                    )
                    nc.scalar.activation(
                        ctr[:, sl], dat[:, sl],
                        mybir.ActivationFunctionType.Square,
                        accum_out=parts[:, col:col + 1],
                    )
                    col += 1
                off += k

            nc.sync.dma_start(out=out_t.ap()[:], in_=parts[:])

    nc.compile()
    return nc


def _get_graph():
    if "nc" not in _cached:
        _cached["nc"] = _build_graph()
    return _cached["nc"]


def _make_in_maps(data, cen, labels):
    data = np.ascontiguousarray(np.asarray(data), dtype=np.float32)
    cen = np.ascontiguousarray(np.asarray(cen), dtype=np.float32)
    labels = np.asarray(labels).astype(np.int32)
    in_maps = []
    for c in range(N_CORES):
        sl = slice(c * B_CORE, (c + 1) * B_CORE)
        in_maps.append(
            {
                "data": data[sl].reshape(P, R * DIM),
                "labels": np.ascontiguousarray(labels[sl].reshape(P, R)),
                "cen": cen,
            }
        )
    return in_maps


def _run(data, cen, labels, trace=False):
    import time

    from concourse.bass_utils import run_bass_kernel_spmd

    nc = _get_graph()
    in_maps = _make_in_maps(data, cen, labels)
    last_err = None
    for attempt in range(3):
        try:
            res = run_bass_kernel_spmd(
                nc, in_maps, core_ids=list(range(N_CORES)), trace=trace
            )
            break
        except Exception as e:  # transient NRT device flakes
            last_err = e
            time.sleep(2.0)
    else:
        raise last_err
    total = float(
        np.sum([res.results[i]["out"].astype(np.float64) for i in range(N_CORES)])
    )
    return np.float32(total / BATCH), res


def kernel(data, cen, labels):
    out, _ = _run(data, cen, labels)
    return out
